# revision 1
# baseline (speedup 1.0000x reference)
"""CenterLoss on 8 Trainium2 NeuronCores (Bass).

reference:
    distmat[b, c] = ||x_b||^2 + ||c_c||^2 - 2<x_b, c_c>          [B, C]
    mask[b, c]    = (labels_b == c)
    loss          = clip(distmat * mask, 1e-12, 1e12).sum() / B

Every masked-out entry of ``distmat * mask`` is exactly 0.0, and
clip(0, 1e-12, 1e12) == 1e-12, so

    loss = ( sum_b clip(||x_b - centers[labels_b]||^2, 1e-12, 1e12)
             + (B*C - B) * 1e-12 ) / B

i.e. only the B gathered center rows are ever needed.  The kernel shards
the batch across the 8 cores (128 rows each); each core indirect-DMA
gathers its 128 center rows from the full centers table in device DRAM,
computes the per-row squared distances on the vector engine, and the host
applies the clip + scalar reduction (plus the closed-form constant from
the clipped zeros).

Raw Bass (no Tile): the walrus build in this container has a very small
per-instruction sync-wait budget, so waits are emitted as standalone
instructions and the Tile epilogue drain/barrier (which aggregates every
semaphore lane into one Drain) is avoided entirely.
"""

import numpy as np

B = 1024
C = 100000
D = 128
NCORES = 8
PB = B // NCORES  # batch rows per core

_CACHE = {}

# Extra kwargs forwarded to run_bass_kernel_spmd (e.g. {"trace": True} from a
# profiling harness).  Empty for normal grading runs.
_RUN_KWARGS = {}


def _build_module():
    import concourse.bass as bass
    import concourse.mybir as mybir

    nc = bass.Bass(name="center_loss_gather")

    # x rows and their labels travel in ONE tensor: column D carries the
    # uint32 label bit-cast to f32, so a single DMA loads both.
    xlab_in = nc.dram_tensor("xlab", [PB, D + 1], mybir.dt.float32, kind="ExternalInput")
    cen_in = nc.dram_tensor("centers", [C, D], mybir.dt.float32, kind="ExternalInput")
    out = nc.dram_tensor("out", [1, 1], mybir.dt.float32, kind="ExternalOutput")

    f32 = mybir.dt.float32
    ones_ap = nc.const_aps.aps[(f32, 1.0)]  # [128,1] preamble constant
    with (
        nc.sbuf_tensor([PB, D + 1], f32) as xlab_t,
        nc.sbuf_tensor([PB, D], f32) as g_t,
        nc.sbuf_tensor([PB, D], f32) as diff_t,
        nc.sbuf_tensor([PB, D], f32) as sq_t,
        nc.sbuf_tensor([1, 1], f32) as sum_sb,
        nc.psum_tensor([1, D], f32) as psum_t,
        nc.semaphore() as in_sem,
        nc.semaphore() as g_sem,
        nc.semaphore() as v_sem,
        nc.semaphore() as pe_sem,
        nc.semaphore() as o_sem,
        nc.Block() as block,
    ):

        @block.sync
        def _(sync):
            sync.dma_start(out=xlab_t[:], in_=xlab_in[:]).then_inc(in_sem, 16)
            # sum_sb holds the final scalar: one contiguous 4-byte store
            # (a [PB,1] per-partition store costs 128 scattered descriptors
            # and a ~6 us completion receipt).  HWDGE via the sync engine so
            # gpsimd's stream (and its epilogue drain) retires early.
            sync.wait_ge(v_sem, 3)
            # No explicit o_sem wait: the Block-exit Drain on this engine
            # quiesces outstanding HWDGE DMAs (observed: the gpsimd epilogue
            # Drain spans exactly until its gather's completion receipt), and
            # it overlaps with the other engines' barrier arrival.
            sync.dma_start(out=out[:], in_=sum_sb[:]).then_inc(o_sem, 16)

        @block.gpsimd
        def _(g):
            g.wait_ge(in_sem, 16)
            g.indirect_dma_start(
                out=g_t[:],
                out_offset=None,
                in_=cen_in[:],
                in_offset=bass.IndirectOffsetOnAxis(
                    ap=xlab_t[:, D : D + 1].bitcast(mybir.dt.uint32),
                    axis=0,
                ),
            ).then_inc(g_sem, 16)

        @block.tensor
        def _(t):
            # Column sums of sq: [1,D] = ones[128,1].T @ sq[128,D].
            t.wait_ge(v_sem, 2)
            t.matmul(
                out=psum_t[:], lhsT=ones_ap, rhs=sq_t[:], start=True, stop=True
            ).then_inc(pe_sem, 1)

        @block.vector
        def _(v):
            v.wait_ge(in_sem, 16)
            v.wait_ge(g_sem, 16)
            v.tensor_sub(out=diff_t[:], in0=xlab_t[:, :D], in1=g_t[:]).then_inc(v_sem, 1)
            v.wait_ge(v_sem, 1)
            v.tensor_mul(out=sq_t[:], in0=diff_t[:], in1=diff_t[:]).then_inc(v_sem, 1)
            v.wait_ge(pe_sem, 1)
            v.reduce_sum(
                out=sum_sb[:], in_=psum_t[:], axis=mybir.AxisListType.X
            ).then_inc(v_sem, 1)

    return nc


def _get_module():
    if "nc" not in _CACHE:
        _CACHE["nc"] = _build_module()
    return _CACHE["nc"]


def kernel(x, labels, centers):
    from concourse.bass_utils import run_bass_kernel_spmd

    x = np.ascontiguousarray(np.asarray(x), dtype=np.float32)
    centers = np.ascontiguousarray(np.asarray(centers), dtype=np.float32)
    labels = np.asarray(labels)
    assert x.shape == (B, D) and centers.shape == (C, D), (x.shape, centers.shape)
    lab_bits = labels.reshape(B, 1).astype(np.uint32).view(np.float32)
    xlab = np.ascontiguousarray(np.concatenate([x, lab_bits], axis=1))

    nc = _get_module()
    in_maps = [
        {
            "xlab": xlab[i * PB : (i + 1) * PB],
            "centers": centers,
        }
        for i in range(NCORES)
    ]
    res = run_bass_kernel_spmd(nc, in_maps, core_ids=list(range(NCORES)), **_RUN_KWARGS)
    _CACHE["last_results"] = res
    # Each core returns the scalar sum of (x - gathered_center)^2 over its
    # 128 rows; the (B*C - B) masked-out zeros clip to exactly 1e-12 each.
    partials = np.array([float(r["out"].reshape(())) for r in res.results])
    total = partials.astype(np.float64).sum() + (B * C - B) * 1e-12
    return np.array(total / B, dtype=np.float32)



# revision 9
# speedup vs baseline: 1.0907x; 1.0907x over previous
"""CenterLoss on 8 Trainium2 NeuronCores (Bass).

reference:
    distmat[b, c] = ||x_b||^2 + ||c_c||^2 - 2<x_b, c_c>          [B, C]
    mask[b, c]    = (labels_b == c)
    loss          = clip(distmat * mask, 1e-12, 1e12).sum() / B

Every masked-out entry of ``distmat * mask`` is exactly 0.0, and
clip(0, 1e-12, 1e12) == 1e-12, so

    loss = ( sum_b clip(||x_b - centers[labels_b]||^2, 1e-12, 1e12)
             + (B*C - B) * 1e-12 ) / B

i.e. only the B gathered center rows are ever needed.  The kernel shards
the batch across the 8 cores (128 rows each); each core indirect-DMA
gathers its 128 center rows from the full centers table in device DRAM,
computes the per-row squared distances on the vector engine, DMAs the
128 per-row sums back, and the host applies the final reduction (plus
the closed-form constant from the clipped zeros).

Raw Bass, no nc.Block(), and the Bass-constructor all_engine_barrier is
elided: the explicit semaphore chain (labels DMA -> gather, x DMA ->
DVE, DVE -> out DMA) already orders every cross-engine dependency, so
the per-engine entry branches, the block-exit barrier and the init
barrier are pure overhead.  The labels ride in their own tiny [128,1]
DMA issued before the x tile so the gather's SWDGE generation starts as
early as possible; offsets MUST be one-per-partition ([128,1] in SBUF)
-- a [1,128] single-partition offset AP compiles but crashes the
device, and DRAM-resident offsets are rejected by walrus.
"""

import numpy as np

B = 1024
C = 100000
D = 128
NCORES = 8
PB = B // NCORES  # batch rows per core

_CACHE = {}

# Extra kwargs forwarded to run_bass_kernel_spmd (e.g. {"trace": True} from a
# profiling harness).  Empty for normal grading runs.
_RUN_KWARGS = {}


def _build_module():
    import concourse.bass as bass
    import concourse.mybir as mybir

    f32 = mybir.dt.float32
    u32 = mybir.dt.uint32

    class FastBass(bass.Bass):
        _in_init = False

        def __init__(self, *a, **k):
            type(self)._in_init = True
            try:
                super().__init__(*a, **k)
            finally:
                type(self)._in_init = False

        def all_engine_barrier(self, *, sem_only: bool = False):
            if type(self)._in_init:
                return
            return super().all_engine_barrier(sem_only=sem_only)

    nc = FastBass(
        name="center_loss_gather",
        enable_partition_id=False,
        monotonic_sem_count=0,
    )

    lab_in = nc.dram_tensor("lab", [PB, 1], u32, kind="ExternalInput")
    x_in = nc.dram_tensor("x", [PB, D], f32, kind="ExternalInput")
    cen_in = nc.dram_tensor("centers", [C, D], f32, kind="ExternalInput")
    out = nc.dram_tensor("out", [PB, 1], f32, kind="ExternalOutput")

    with (
        nc.sbuf_tensor([PB, 1], u32) as lab_t,
        nc.sbuf_tensor([PB, D], f32) as x_t,
        nc.sbuf_tensor([PB, D], f32) as g_t,
        nc.sbuf_tensor([PB, D], f32) as diff_t,
        nc.sbuf_tensor([PB, D], f32) as sq_t,
        nc.sbuf_tensor([PB, 1], f32) as rsum_t,
        nc.semaphore() as l_sem,
        nc.semaphore() as x_sem,
        nc.semaphore() as g_sem,
        nc.semaphore() as v_sem,
        nc.semaphore() as o_sem,
    ):
        sp = nc.sync
        gp = nc.gpsimd
        v = nc.vector

        # Labels first: 128 4-byte descriptors clear the queue in ~60ns,
        # so the gather's semaphore fires ~130ns before a merged x+labels
        # load would allow, and the 66KB x tile streams in parallel with
        # the SWDGE descriptor generation.
        sp.dma_start(out=lab_t[:], in_=lab_in[:]).then_inc(l_sem, 16)
        sp.dma_start(out=x_t[:], in_=x_in[:]).then_inc(x_sem, 16)

        gp.wait_ge(l_sem, 16)
        gp.indirect_dma_start(
            out=g_t[:],
            out_offset=None,
            in_=cen_in[:],
            in_offset=bass.IndirectOffsetOnAxis(ap=lab_t[:], axis=0),
        ).then_inc(g_sem, 16)

        v.wait_ge(x_sem, 16)
        v.wait_ge(g_sem, 16)
        v.tensor_sub(out=diff_t[:], in0=x_t[:], in1=g_t[:])
        # (tensor_tensor_reduce would fuse these two, but walrus in this
        # toolchain rejects InstTensorTensorReduce with "ISA wrong length".)
        v.tensor_mul(out=sq_t[:], in0=diff_t[:], in1=diff_t[:])
        v.reduce_sum(
            out=rsum_t[:], in_=sq_t[:], axis=mybir.AxisListType.X
        ).then_inc(v_sem, 1)

        # One [128,1] store of the row sums; the host finishes the
        # reduction.  Issued from SP: its sem wake is the fastest
        # (SEM_PROP_RECV[SP,SEQ]=0) and its DMA path beats DVE's
        # (565+650 vs 667+784 ns of issue+DGE latency).  Every DMA must
        # carry a sem update ("DGE must have sync info"); o_sem is never
        # waited on -- the drain covers completion before the NEFF end.
        sp.wait_ge(v_sem, 1)
        sp.dma_start(out=out[:], in_=rsum_t[:]).then_inc(o_sem, 16)
        sp.drain()

    return nc


def _get_module():
    if "nc" not in _CACHE:
        _CACHE["nc"] = _build_module()
    return _CACHE["nc"]


def kernel(x, labels, centers):
    from concourse.bass_utils import run_bass_kernel_spmd

    x = np.ascontiguousarray(np.asarray(x), dtype=np.float32)
    centers = np.ascontiguousarray(np.asarray(centers), dtype=np.float32)
    labels = np.asarray(labels)
    assert x.shape == (B, D) and centers.shape == (C, D), (x.shape, centers.shape)
    lab_u32 = np.ascontiguousarray(labels.reshape(B, 1).astype(np.uint32))

    nc = _get_module()
    in_maps = [
        {
            "lab": lab_u32[i * PB : (i + 1) * PB],
            "x": x[i * PB : (i + 1) * PB],
            "centers": centers,
        }
        for i in range(NCORES)
    ]
    res = run_bass_kernel_spmd(nc, in_maps, core_ids=list(range(NCORES)), **_RUN_KWARGS)
    _CACHE["last_results"] = res
    # Each core returns its 128 per-row squared distances; the (B*C - B)
    # masked-out zeros clip to exactly 1e-12 each.
    partials = np.concatenate([r["out"].reshape(-1) for r in res.results])
    total = partials.astype(np.float64).sum() + (B * C - B) * 1e-12
    return np.array(total / B, dtype=np.float32)
